# revision 12
# baseline (speedup 1.0000x reference)
"""Channel-transformer (CTR) attention kernel for Trainium2, 8 NeuronCores.

Problem: x (16, 256, 64, 64) f32, gamma scalar.
  xr = x.reshape(B, C, NH, DIM)                       # NH=8, DIM=512
  energy[b,h,c,k] = sum_d xr[b,c,h,d] * xr[b,k,h,d]   # symmetric (C x C)
  attn = softmax(rowmax(energy) - energy, axis=-1)    # == softmax(-energy)
  out[b,c,h,d] = sum_k attn[b,h,c,k] * xr[b,k,h,d]
  result = gamma * out + x

Sharding: data-parallel over batch, 2 samples per core; gamma replicated.

I/O is bf16: host casts x to bf16 (matmuls are bf16 anyway; the residual in
bf16 costs ~1e-3 rel err, well inside tolerance) and y is stored bf16 and
upcast on host.  This halves HBM traffic vs f32 so the PE becomes the
bottleneck (compute regime).

Per-core kernel (16 units = 2 samples x 8 heads), software-pipelined so the
PE stream stays dense:
  S1 PE   : 8 transposes of 128x128 XB blocks -> tp PSUM [128,1024] bf16
  S2 Pool : copy tp -> XT SBUF
  S3 PE   : E[m] = XT[:,m-half].T @ XT (8 matmuls, f32 PSUM)
  S4 ACT  : attnT[kc] = exp(-E[kc] - 64) bf16, accum_out = Z (softmax denom;
            the max-shift cancels row-wise so a constant bias suffices, and
            E symmetric => these tiles are already k-major attnT)
  S5 PE   : V[m] += attnT[kc][:,m-half].T @ XB[kc][:,head] (4 matmuls)
  S6 DVE/Pool: st[:,m*512:] = V[m]*(gamma/Z) + XB[m][:,head]
  S7 SP   : one merged DMA per unit stores both m-halves of y
PE program order per round r: S1(r), S3(r-1), S5(r-2) so transposes of the
next unit fill the gap while Pool/ACT produce this unit's inputs.
"""

import numpy as np

B, C, HW = 16, 256, 4096
NH, DIM = 8, 512
N_CORES = 8
BPC = B // N_CORES  # batches per core
NU = BPC * NH  # pipelined units per core
EXP_BIAS = -64.0  # exp(-E + EXP_BIAS): keeps exponents < ~40 for N(0,1) inputs

_CACHE = {}


def _build_module():
    import os
    import concourse.bacc as bacc
    import concourse.tile as tile
    import concourse.mybir as mybir

    f32 = mybir.dt.float32
    bf16 = mybir.dt.bfloat16
    AF = mybir.ActivationFunctionType
    OP = mybir.AluOpType

    nc = bacc.Bacc("TRN2", target_bir_lowering=False, debug=False, num_devices=N_CORES)
    x_d = nc.dram_tensor("x", [BPC, 2, 128, HW], bf16, kind="ExternalInput").ap()
    g_d = nc.dram_tensor("g", [1, 1], f32, kind="ExternalInput").ap()
    id_d = nc.dram_tensor("ident", [128, 128], bf16, kind="ExternalInput").ap()
    y_d = nc.dram_tensor("y", [BPC, 2, 128, HW], bf16, kind="ExternalOutput").ap()

    _warm = int(os.environ.get("K_WARM", "16"))

    with tile.TileContext(nc) as tc:
        from contextlib import ExitStack

        with ExitStack() as ctx:
            xb_pool = ctx.enter_context(tc.tile_pool(name="xb", bufs=2 * BPC))
            st_pool = ctx.enter_context(tc.tile_pool(name="st", bufs=3))
            xt_pool = ctx.enter_context(tc.tile_pool(name="xt", bufs=3))
            at_pool = ctx.enter_context(tc.tile_pool(name="at", bufs=6))
            r_pool = ctx.enter_context(tc.tile_pool(name="rp", bufs=6))
            tp_pool = ctx.enter_context(tc.tile_pool(name="tp", bufs=2, space="PSUM"))
            e_pool = ctx.enter_context(tc.tile_pool(name="pe", bufs=2, space="PSUM"))
            ev_pool = ctx.enter_context(tc.tile_pool(name="ev", bufs=4, space="PSUM"))

            cpool = ctx.enter_context(tc.tile_pool(name="const", bufs=1))
            ident = cpool.tile([128, 128], bf16)
            # ident/g on the ACT HWDGE queue so they are not stuck behind
            # the big x loads (warmups depend on ident).
            nc.scalar.dma_start(ident[:], id_d[:])
            ebias = cpool.tile([128, 1], f32)
            nc.gpsimd.memset(ebias[:], EXP_BIAS)
            onesr = cpool.tile([1, 128], f32)
            nc.gpsimd.memset(onesr[:], 1.0)
            gsb = cpool.tile([1, 1], f32)
            nc.scalar.dma_start(gsb[:], g_d[:])
            gamma128 = cpool.tile([128, 1], f32)
            # broadcast gamma to all partitions: [128,1] = ones[1,128].T @ g[1,1]
            gps = ev_pool.tile([128, 1], f32, tag="ev", name="gps")
            nc.tensor.matmul(gps[:], onesr[:], gsb[:], start=True, stop=True)
            nc.scalar.copy(gamma128[:], gps[:])

            warm = e_pool.tile([128, 512], f32, tag="pe", name="warm")
            for _w in range(_warm):
                nc.tensor.matmul(warm[0:64, 0:64], ident[0:64, 0:64], ident[0:64, 0:64], start=True, stop=True)

            # ---- input loads: bf16, chunked so unit 0 starts early.
            # b0 on the ACT queue (right behind ident), b1 on SP. ----
            XBall = []
            for b in range(BPC):
                XB = [xb_pool.tile([128, HW], bf16, tag="xb", name=f"XB{b}_{m}") for m in range(2)]
                dma_eng = nc.scalar if b == 0 else nc.sync
                for c0, c1 in ((0, 1024), (1024, 4096)):
                    for m in range(2):
                        dma_eng.dma_start(XB[m][:, c0:c1], x_d[b, m, :, c0:c1])
                XBall.append(XB)

            # ---- software-pipelined unit stages ----
            tps = [None] * NU
            XTs = [None] * NU
            Es = [None] * NU
            ATs = [None] * NU
            Zps = [None] * NU
            gRps = [None] * NU
            Vs = [None] * NU
            sts = [None] * NU

            def S1_transpose(u):
                b, h = divmod(u, NH)
                XB = XBall[b]
                col = DIM * h
                tp = tp_pool.tile([128, 1024], bf16, tag="tp", name=f"TP{u}")
                for kd in range(4):
                    for m in range(2):
                        nc.tensor.transpose(
                            tp[:, 256 * kd + 128 * m : 256 * kd + 128 * (m + 1)],
                            XB[m][:, col + 128 * kd : col + 128 * (kd + 1)],
                            ident[:],
                        )
                tps[u] = tp

            def S2_copy(u):
                # PSUM->SBUF move of XT; bf16 pairs copied as f32 words to
                # halve engine cycles; alternate ACT/DVE per unit to balance.
                # (GPSIMD cannot access PSUM on TRN2.)
                XT = xt_pool.tile([128, 1024], bf16, tag="xt", name=f"XT{u}")
                src = tps[u][:].bitcast(f32)
                dst = XT[:].bitcast(f32)
                if u % 2 == 0:
                    nc.scalar.copy(dst[:], src[:])
                else:
                    nc.vector.tensor_copy(dst[:], src[:])
                XTs[u] = XT

            def S3_energy(u):
                XT = XTs[u]
                E = e_pool.tile([128, 512], f32, tag="pe", name=f"E{u}")
                for m in range(2):
                    for kd in range(4):
                        nc.tensor.matmul(
                            E[:, 256 * m : 256 * (m + 1)],
                            XT[:, 256 * kd + 128 * m : 256 * kd + 128 * (m + 1)],
                            XT[:, 256 * kd : 256 * (kd + 1)],
                            start=(kd == 0),
                            stop=(kd == 3),
                        )
                Es[u] = E

            def S4_exp(u):
                E = Es[u]
                AT = []
                # one tile for Zp|Rp|gRp: fewer tiles -> fewer semaphores
                zr = r_pool.tile([128, 6], f32, tag="rp", name=f"ZR{u}")
                for kc in range(2):
                    a = at_pool.tile([128, 256], bf16, tag="at", name=f"AT{u}_{kc}")
                    nc.scalar.activation(
                        a[:], E[:, 256 * kc : 256 * (kc + 1)], AF.Exp,
                        scale=-1.0, bias=ebias[:], accum_out=zr[:, kc : kc + 1],
                    )
                    AT.append(a)
                ATs[u] = AT
                # keep the whole Z -> gamma/Z chain on DVE (no Pool hop)
                nc.vector.reciprocal(zr[:, 2:4], zr[:, 0:2])
                nc.vector.tensor_scalar(zr[:, 4:6], zr[:, 2:4], gamma128[:], None, op0=OP.mult)
                gRps[u] = zr

            def S5_apply(u):
                b, h = divmod(u, NH)
                XB = XBall[b]
                col = DIM * h
                AT = ATs[u]
                V = [ev_pool.tile([128, DIM], f32, tag="ev", name=f"V{u}_{m}") for m in range(2)]
                for m in range(2):
                    for kc in range(2):
                        nc.tensor.matmul(
                            V[m][:],
                            AT[kc][:, 128 * m : 128 * (m + 1)],
                            XB[kc][:, col : col + DIM],
                            start=(kc == 0),
                            stop=(kc == 1),
                        )
                Vs[u] = V

            def S6_scale(u):
                b, h = divmod(u, NH)
                XB = XBall[b]
                col = DIM * h
                V, zr = Vs[u], gRps[u]
                # two heads share one staging tile: [m0:h_ev|h_od, m1:h_ev|h_od]
                if u % 2 == 0:
                    sts[u] = st_pool.tile([128, 2048], bf16, tag="st", name=f"ST{u}")
                st = sts[u // 2 * 2]
                p = u % 2
                for m in range(2):
                    nc.vector.scalar_tensor_tensor(
                        st[:, 1024 * m + 512 * p : 1024 * m + 512 * (p + 1)],
                        V[m][:],
                        zr[:, 4 + m : 5 + m],
                        XB[m][:, col : col + DIM],
                        op0=OP.mult,
                        op1=OP.add,
                    )

            def S7_store(u):
                # one 2D store per (head-pair, m): 2KB lines, spread across
                # DMA engines (3D merged stores serialize onto 1-2 engines).
                if u % 2 == 0:
                    return
                b, h = divmod(u, NH)
                col = DIM * (h - 1)
                st = sts[u - 1]
                for m in range(2):
                    nc.sync.dma_start(
                        y_d[b, m, :, col : col + 2 * DIM],
                        st[:, 1024 * m : 1024 * (m + 1)],
                    )

            for r in range(NU + 2):
                if r < NU:
                    S1_transpose(r)
                    S2_copy(r)
                if 0 <= r - 1 < NU:
                    S3_energy(r - 1)
                    S4_exp(r - 1)
                if 0 <= r - 2 < NU:
                    S5_apply(r - 2)
                    S6_scale(r - 2)
                    S7_store(r - 2)

    nc.compile()
    return nc


def _get_module():
    if "nc" not in _CACHE:
        _CACHE["nc"] = _build_module()
    return _CACHE["nc"]


def _make_in_maps(x_input, gamma):
    import ml_dtypes

    x = np.ascontiguousarray(np.asarray(x_input, dtype=np.float32)).reshape(
        N_CORES, BPC, 2, 128, HW
    )
    xb = x.astype(ml_dtypes.bfloat16)
    g = np.asarray(gamma, dtype=np.float32).reshape(1, 1)
    ident = np.eye(128, dtype=ml_dtypes.bfloat16)
    return [
        {"x": np.ascontiguousarray(xb[i]), "g": g, "ident": ident}
        for i in range(N_CORES)
    ]


def kernel(x_input, gamma):
    from concourse.bass_utils import run_bass_kernel_spmd

    nc = _get_module()
    in_maps = _make_in_maps(x_input, gamma)
    res = run_bass_kernel_spmd(nc, in_maps, list(range(N_CORES)))
    y = np.stack([np.asarray(res.results[i]["y"]) for i in range(N_CORES)], axis=0)
    return y.astype(np.float32).reshape(B, C, 64, 64)


# revision 15
# speedup vs baseline: 1.1664x; 1.1664x over previous
"""Channel-transformer (CTR) attention kernel for Trainium2, 8 NeuronCores.

Problem: x (16, 256, 64, 64) f32, gamma scalar.
  xr = x.reshape(B, C, NH, DIM)                       # NH=8, DIM=512
  energy[b,h,c,k] = sum_d xr[b,c,h,d] * xr[b,k,h,d]   # symmetric (C x C)
  attn = softmax(rowmax(energy) - energy, axis=-1)    # == softmax(-energy)
  out[b,c,h,d] = sum_k attn[b,h,c,k] * xr[b,k,h,d]
  result = gamma * out + x

Sharding: data-parallel over batch, 2 samples per core; gamma replicated.

I/O is bf16: host casts x to bf16 (matmuls are bf16 anyway; the residual in
bf16 costs ~1e-3 rel err, well inside tolerance) and y is stored bf16 and
upcast on host.  This halves HBM traffic vs f32 so the PE becomes the
bottleneck (compute regime).

Per-core kernel (16 units = 2 samples x 8 heads), software-pipelined so the
PE stream stays dense:
  S1 PE   : 8 transposes of 128x128 XB blocks -> tp PSUM [128,1024] bf16
  S2 Pool : copy tp -> XT SBUF
  S3 PE   : E[m] = XT[:,m-half].T @ XT (8 matmuls, f32 PSUM)
  S4 ACT  : attnT[kc] = exp(-E[kc] - 64) bf16, accum_out = Z (softmax denom;
            the max-shift cancels row-wise so a constant bias suffices, and
            E symmetric => these tiles are already k-major attnT)
  S5 PE   : V[m] += attnT[kc][:,m-half].T @ XB[kc][:,head] (4 matmuls)
  S6 DVE/Pool: st[:,m*512:] = V[m]*(gamma/Z) + XB[m][:,head]
  S7 SP   : one merged DMA per unit stores both m-halves of y
PE program order per round r: S1(r), S3(r-1), S5(r-2) so transposes of the
next unit fill the gap while Pool/ACT produce this unit's inputs.
"""

import numpy as np

B, C, HW = 16, 256, 4096
NH, DIM = 8, 512
N_CORES = 8
BPC = B // N_CORES  # batches per core
NU = BPC * NH  # pipelined units per core
EXP_BIAS = -64.0  # exp(-E + EXP_BIAS): keeps exponents < ~40 for N(0,1) inputs

_CACHE = {}


def _build_module():
    import os
    import concourse.bacc as bacc
    import concourse.tile as tile
    import concourse.mybir as mybir

    f32 = mybir.dt.float32
    bf16 = mybir.dt.bfloat16
    AF = mybir.ActivationFunctionType
    OP = mybir.AluOpType

    nc = bacc.Bacc("TRN2", target_bir_lowering=False, debug=False, num_devices=N_CORES)
    x_d = nc.dram_tensor("x", [BPC, 2, 128, HW], bf16, kind="ExternalInput").ap()
    g_d = nc.dram_tensor("g", [1, 1], f32, kind="ExternalInput").ap()
    y_d = nc.dram_tensor("y", [BPC, 2, 128, HW], bf16, kind="ExternalOutput").ap()

    _warm = int(os.environ.get("K_WARM", "36"))

    with tile.TileContext(nc) as tc:
        from contextlib import ExitStack

        with ExitStack() as ctx:
            xb_pool = ctx.enter_context(tc.tile_pool(name="xb", bufs=2 * BPC))
            st_pool = ctx.enter_context(tc.tile_pool(name="st", bufs=3))
            xt_pool = ctx.enter_context(tc.tile_pool(name="xt", bufs=3))
            at_pool = ctx.enter_context(tc.tile_pool(name="at", bufs=6))
            r_pool = ctx.enter_context(tc.tile_pool(name="rp", bufs=6))
            tp_pool = ctx.enter_context(tc.tile_pool(name="tp", bufs=2, space="PSUM"))
            e_pool = ctx.enter_context(tc.tile_pool(name="pe", bufs=2, space="PSUM"))
            ev_pool = ctx.enter_context(tc.tile_pool(name="ev", bufs=4, space="PSUM"))

            cpool = ctx.enter_context(tc.tile_pool(name="const", bufs=1))
            # identity built on-chip (memset ones + affine_select f==p):
            # avoids a DRAM input whose ~4us DMA latency gated the warmups.
            wones = cpool.tile([128, 128], bf16)
            nc.gpsimd.memset(wones[:], 1.0)
            ident = cpool.tile([128, 128], bf16)
            nc.gpsimd.affine_select(
                ident[:], wones[:], pattern=[[1, 128]],
                compare_op=mybir.AluOpType.is_equal, fill=0.0,
                base=0, channel_multiplier=-1,
            )
            ebias = cpool.tile([128, 1], f32)
            nc.gpsimd.memset(ebias[:], EXP_BIAS)
            onesr = cpool.tile([1, 128], f32)
            nc.gpsimd.memset(onesr[:], 1.0)
            gsb = cpool.tile([1, 1], f32)
            nc.scalar.dma_start(gsb[:], g_d[:])
            gamma128 = cpool.tile([128, 1], f32)
            # broadcast gamma to all partitions: [128,1] = ones[1,128].T @ g[1,1]
            gps = ev_pool.tile([128, 1], f32, tag="ev", name="gps")
            nc.tensor.matmul(gps[:], onesr[:], gsb[:], start=True, stop=True)
            nc.scalar.copy(gamma128[:], gps[:])

            warm = e_pool.tile([128, 512], f32, tag="pe", name="warm")
            for _w in range(_warm):
                nc.tensor.matmul(warm[0:64, 0:64], wones[0:64, 0:64], wones[0:64, 0:64], start=True, stop=True)

            # ---- input loads: bf16, all on SP. b0 in fine chunks so the
            # pipeline is never starved (DMA deps are per-instruction). ----
            XBall = []
            for b in range(BPC):
                XB = [xb_pool.tile([128, HW], bf16, tag="xb", name=f"XB{b}_{m}") for m in range(2)]
                chunks = ((0, 1024), (1024, 2048), (2048, 3072), (3072, 4096)) if b == 0 else ((0, 2048), (2048, 4096))
                for c0, c1 in chunks:
                    for m in range(2):
                        nc.sync.dma_start(XB[m][:, c0:c1], x_d[b, m, :, c0:c1])
                XBall.append(XB)

            # ---- software-pipelined unit stages ----
            tps = [None] * NU
            XTs = [None] * NU
            Es = [None] * NU
            ATs = [None] * NU
            Zps = [None] * NU
            gRps = [None] * NU
            Vs = [None] * NU
            sts = [None] * NU

            def S1_transpose(u):
                b, h = divmod(u, NH)
                XB = XBall[b]
                col = DIM * h
                tp = tp_pool.tile([128, 1024], bf16, tag="tp", name=f"TP{u}")
                for kd in range(4):
                    for m in range(2):
                        nc.tensor.transpose(
                            tp[:, 256 * kd + 128 * m : 256 * kd + 128 * (m + 1)],
                            XB[m][:, col + 128 * kd : col + 128 * (kd + 1)],
                            ident[:],
                        )
                tps[u] = tp

            def S2_copy(u):
                # PSUM->SBUF move of XT; bf16 pairs copied as f32 words to
                # halve engine cycles; alternate ACT/DVE per unit to balance.
                # (GPSIMD cannot access PSUM on TRN2.)
                XT = xt_pool.tile([128, 1024], bf16, tag="xt", name=f"XT{u}")
                src = tps[u][:].bitcast(f32)
                dst = XT[:].bitcast(f32)
                if u % 2 == 0:
                    nc.scalar.copy(dst[:], src[:])
                else:
                    nc.vector.tensor_copy(dst[:], src[:])
                XTs[u] = XT

            def S3_energy(u):
                XT = XTs[u]
                E = e_pool.tile([128, 512], f32, tag="pe", name=f"E{u}")
                for m in range(2):
                    for kd in range(4):
                        nc.tensor.matmul(
                            E[:, 256 * m : 256 * (m + 1)],
                            XT[:, 256 * kd + 128 * m : 256 * kd + 128 * (m + 1)],
                            XT[:, 256 * kd : 256 * (kd + 1)],
                            start=(kd == 0),
                            stop=(kd == 3),
                        )
                Es[u] = E

            def S4_exp(u):
                E = Es[u]
                AT = []
                # one tile for Zp|Rp|gRp: fewer tiles -> fewer semaphores
                zr = r_pool.tile([128, 6], f32, tag="rp", name=f"ZR{u}")
                for kc in range(2):
                    a = at_pool.tile([128, 256], bf16, tag="at", name=f"AT{u}_{kc}")
                    nc.scalar.activation(
                        a[:], E[:, 256 * kc : 256 * (kc + 1)], AF.Exp,
                        scale=-1.0, bias=ebias[:], accum_out=zr[:, kc : kc + 1],
                    )
                    AT.append(a)
                ATs[u] = AT
                # keep the whole Z -> gamma/Z chain on DVE (no Pool hop)
                nc.vector.reciprocal(zr[:, 2:4], zr[:, 0:2])
                nc.vector.tensor_scalar(zr[:, 4:6], zr[:, 2:4], gamma128[:], None, op0=OP.mult)
                gRps[u] = zr

            def S5_apply(u):
                b, h = divmod(u, NH)
                XB = XBall[b]
                col = DIM * h
                AT = ATs[u]
                V = [ev_pool.tile([128, DIM], f32, tag="ev", name=f"V{u}_{m}") for m in range(2)]
                for m in range(2):
                    for kc in range(2):
                        nc.tensor.matmul(
                            V[m][:],
                            AT[kc][:, 128 * m : 128 * (m + 1)],
                            XB[kc][:, col : col + DIM],
                            start=(kc == 0),
                            stop=(kc == 1),
                        )
                Vs[u] = V

            def S6_scale(u):
                b, h = divmod(u, NH)
                XB = XBall[b]
                col = DIM * h
                V, zr = Vs[u], gRps[u]
                # two heads share one staging tile: [m0:h_ev|h_od, m1:h_ev|h_od]
                if u % 2 == 0:
                    sts[u] = st_pool.tile([128, 2048], bf16, tag="st", name=f"ST{u}")
                st = sts[u // 2 * 2]
                p = u % 2
                for m in range(2):
                    nc.vector.scalar_tensor_tensor(
                        st[:, 1024 * m + 512 * p : 1024 * m + 512 * (p + 1)],
                        V[m][:],
                        zr[:, 4 + m : 5 + m],
                        XB[m][:, col : col + DIM],
                        op0=OP.mult,
                        op1=OP.add,
                    )

            def S7_store(u):
                # one 2D store per (head-pair, m): 2KB lines, spread across
                # DMA engines (3D merged stores serialize onto 1-2 engines).
                if u % 2 == 0:
                    return
                b, h = divmod(u, NH)
                col = DIM * (h - 1)
                st = sts[u - 1]
                for m in range(2):
                    nc.sync.dma_start(
                        y_d[b, m, :, col : col + 2 * DIM],
                        st[:, 1024 * m : 1024 * (m + 1)],
                    )

            for r in range(NU + 2):
                if r < NU:
                    S1_transpose(r)
                    S2_copy(r)
                if 0 <= r - 1 < NU:
                    S3_energy(r - 1)
                    S4_exp(r - 1)
                if 0 <= r - 2 < NU:
                    S5_apply(r - 2)
                    S6_scale(r - 2)
                    S7_store(r - 2)

    nc.compile()
    return nc


def _get_module():
    if "nc" not in _CACHE:
        _CACHE["nc"] = _build_module()
    return _CACHE["nc"]


def _make_in_maps(x_input, gamma):
    import ml_dtypes

    x = np.ascontiguousarray(np.asarray(x_input, dtype=np.float32)).reshape(
        N_CORES, BPC, 2, 128, HW
    )
    xb = x.astype(ml_dtypes.bfloat16)
    g = np.asarray(gamma, dtype=np.float32).reshape(1, 1)
    return [
        {"x": np.ascontiguousarray(xb[i]), "g": g}
        for i in range(N_CORES)
    ]


def kernel(x_input, gamma):
    from concourse.bass_utils import run_bass_kernel_spmd

    nc = _get_module()
    in_maps = _make_in_maps(x_input, gamma)
    res = run_bass_kernel_spmd(nc, in_maps, list(range(N_CORES)))
    y = np.stack([np.asarray(res.results[i]["y"]) for i in range(N_CORES)], axis=0)
    return y.astype(np.float32).reshape(B, C, 64, 64)
